# revision 25
# baseline (speedup 1.0000x reference)
"""Multi-head causal attention (B=4, L=2048, D=1024, H=16, dh=64) on 8 TRN2 NeuronCores.

Sharding: core i handles batch b = i//2 and head-group g = i%2 (8 heads each).
No cross-core communication needed: each core computes o[b, :, g*512:(g+1)*512].

Per-core dataflow (all layouts chosen so matmul contraction is on partitions):
  inputs (host-prepared, bf16, tiled so every DMA is a single contiguous ~1MB read):
    qTt/kTt/vTt [512, 4096]: row lb*128+p, col d*512+c  holds  x[b][lb*512+c, d*128+p]
    wq/wk/wv    [128, 4096]: row p,       col d*512+c  holds  W[d*128+p, c]
  projections (bf16 matmuls, fp32 psum):
    qwT/kwT [128(2 heads x 64dh), L] bf16;  vw_aug [128(Lk sub), 8*65] bf16 with a
    v_mask column appended per head (gives sum-of-exp for free in the PV matmul).
  attention, per (q-tile tau of 512, head h, k-chunk of 4 subtiles):
    S^T[k,q] = kwT.T @ qwT per 128-k subtile (K=64 on the PE),
    P^T = exp(S^T/8) via ScalarE (psum->sbuf, bf16), causal zeroing of the
    partial 128x128 diagonal triangle via a DVE multiply with one shared
    triangular mask, then O[q, 65] += P^T-block.T @ vw_aug accumulated over
    k-subtiles directly in [q-partition, head-dim] orientation (col 64 = sum
    of exp).  No PE transposes needed; rows are scaled by 1/sumexp on DVE.
    k-chunks of a q-tile spill partial sums to SBUF (flash-attention style)
    so chunks can be scheduled early, keeping the exp load on ScalarE flat
    across the whole kernel while projections fill the PE slack.
v_mask is pre-applied to v on host (and to the ones column via vmask_t on device);
q_mask is applied to the returned output on host.  Masks are {0,1} so this is exact.
"""
import numpy as np
import ml_dtypes
from contextlib import ExitStack

import concourse.bass as bass
import concourse.tile as tile
from concourse import bacc, mybir
from concourse.bass_utils import run_bass_kernel_spmd

F32 = mybir.dt.float32
BF16 = mybir.dt.bfloat16
BF16_NP = ml_dtypes.bfloat16

L = 2048          # sequence length
D = 1024          # d_model
COLS = 512        # projection columns per core (8 heads x 64)
NKSUB = L // 128  # 16 k-subtiles
NTAU = L // 512   # 4 q-tiles
NH = 8            # heads per core


def _default_plan():
    """Emission plan: list of ops.
    ('pre',)                    startup DMA sequence (interleaved halves)
    ('proj', t, lb, i)          queue one projection part (2x4 matmuls)
    ('attn', tau, h, ulo, uhi)  one attention chunk (k-subtiles ulo..uhi)
    ('store', tau)              DMA the finished q-tile out

    Proj parts are queued; each attention chunk first emits any queued parts
    it depends on, then drips the rest into its batch slots (half a part per
    exp batch) so the PE stream never blocks the S matmuls that feed ScalarE.
    """
    def P(t, lb, i):
        return ("proj", t, lb, i)

    plan = [("pre",), P("k", 0, 0), P("q", 0, 0),
            P("v", 0, 0), P("v", 0, 1), P("v", 0, 2), P("v", 0, 3),
            ("load", "q", 1)]

    def wave(tau, ulo, uhi, pre=None):
        for h in range(NH):
            if pre and h in pre:
                plan.extend(pre[h])
            plan.append(("attn", tau, h, ulo, uhi))

    wave(0, 0, 3, pre={2: [P("k", 0, 1), P("q", 0, 1)],
                       3: [("load", "q", 2)],
                       4: [P("k", 0, 2), P("q", 0, 2)],
                       6: [P("k", 0, 3), P("q", 0, 3)],
                       7: [("load", "k", 1), ("load", "v", 1)]})
    wave(1, 0, 3, pre={h: [P("q", 1, h // 2)] for h in (0, 2, 4, 6)})
    plan.append(("load", "q", 3))
    wave(2, 0, 3, pre={h: [P("q", 2, h // 2)] for h in (0, 2, 4, 6)})
    plan.append(("load", "k", 2))
    wave(3, 0, 3, pre={0: [P("q", 3, 0), P("v", 1, 0)],
                       2: [P("q", 3, 1), P("v", 1, 1)],
                       4: [P("q", 3, 2), P("v", 1, 2)],
                       6: [P("q", 3, 3), P("v", 1, 3)]})
    wave(1, 4, 7, pre={0: [P("k", 1, 0)],
                       2: [P("k", 1, 1), ("load", "v", 2)],
                       4: [P("k", 1, 2)],
                       6: [P("k", 1, 3)]})
    wave(2, 4, 7, pre={0: [P("k", 2, 0)],
                       2: [P("k", 2, 1), ("load", "k", 3)],
                       4: [P("k", 2, 2)],
                       6: [P("k", 2, 3)]})
    wave(3, 4, 7, pre={h: [P("v", 2, h // 2)] for h in (0, 2, 4, 6)})
    wave(2, 8, 11, pre={0: [P("k", 3, 0)],
                        2: [P("k", 3, 1), ("load", "v", 3)],
                        4: [P("k", 3, 2)],
                        6: [P("k", 3, 3)]})
    wave(3, 8, 11, pre={0: [P("v", 3, 0), P("v", 3, 1)],
                        4: [P("v", 3, 2), P("v", 3, 3)]})
    wave(3, 12, 15)
    return plan


def _build_kernel(reps=1, plan=None):
    if plan is None:
        plan = _default_plan()
    nc = bacc.Bacc("TRN2", target_bir_lowering=False, debug=False, num_devices=8)

    qTt = nc.dram_tensor("qTt", [512, 4096], BF16, kind="ExternalInput").ap()
    kTt = nc.dram_tensor("kTt", [512, 4096], BF16, kind="ExternalInput").ap()
    vTt = nc.dram_tensor("vTt", [512, 4096], BF16, kind="ExternalInput").ap()
    wq = nc.dram_tensor("wq", [128, 4096], BF16, kind="ExternalInput").ap()
    wk = nc.dram_tensor("wk", [128, 4096], BF16, kind="ExternalInput").ap()
    wv = nc.dram_tensor("wv", [128, 4096], BF16, kind="ExternalInput").ap()
    vmask_t = nc.dram_tensor("vmask_t", [128, NKSUB], F32, kind="ExternalInput").ap()
    out = nc.dram_tensor("out", [L, COLS], F32, kind="ExternalOutput").ap()

    w_src = {"q": wq, "k": wk, "v": wv}
    a_src = {"q": qTt, "k": kTt, "v": vTt}

    with tile.TileContext(nc) as tc, ExitStack() as ctx:
        sb = ctx.enter_context(tc.tile_pool(name="sb", bufs=1))
        ps = ctx.enter_context(tc.tile_pool(name="ps", bufs=1, space="PSUM"))

        # ---- persistent SBUF tensors ----
        w_t = {t: sb.tile([128, 4096], BF16, tag="w", bufs=3, name=f"w{t}")
               for t in ("q", "k", "v")}

        vmask_sb = sb.tile([128, NKSUB], F32, tag="vm")
        nc.sync.dma_start(vmask_sb[:], vmask_t[:])

        # shared causal triangle for the partial diagonal blocks:
        # tri[p, c] = 1 if c >= p else 0   (query col >= key partition)
        tri = sb.tile([128, 128], BF16, tag="tri")
        nc.gpsimd.memset(tri[:], 1.0)
        nc.gpsimd.affine_select(
            out=tri[:], in_=tri[:], compare_op=mybir.AluOpType.is_ge, fill=0.0,
            base=0, channel_multiplier=-1, pattern=[[1, 128]])

        qwT = [sb.tile([128, L], BF16, tag="qwT", bufs=4, name=f"qwT{hp}")
               for hp in range(4)]
        kwT = [sb.tile([128, L], BF16, tag="kwT", bufs=4, name=f"kwT{hp}")
               for hp in range(4)]
        vw_aug = [sb.tile([128, NH * 65], BF16, tag="vwa", bufs=NKSUB,
                          name=f"vwa{u}") for u in range(NKSUB)]

        state = {}

        def get_oo(tau):
            key = ("oo", tau)
            if key not in state:
                state[key] = sb.tile([128, 4 * COLS], F32, tag="oo", bufs=2,
                                     name=f"oo{tau}")
            return state[key]

        def dma_w(tname, c0, c1):
            key = ("w", tname, c0)
            if key not in state:
                state[key] = True
                nc.sync.dma_start(w_t[tname][:, c0:c1], w_src[tname][:, c0:c1])

        def dma_a(tname, lb, c0, c1):
            key = ("adma", tname, lb, c0)
            if key in state:
                return
            state[key] = True
            a = get_act_tile(tname, lb)
            nc.sync.dma_start(a[:, c0:c1],
                              a_src[tname][lb * 128:(lb + 1) * 128, c0:c1])

        def get_act_tile(tname, lb):
            key = ("act", tname, lb)
            if key not in state:
                state[key] = sb.tile([128, 4096], BF16, tag="act", bufs=6,
                                     name=f"a{tname}{lb}")
            return state[key]

        def prologue_dma():
            # interleave so the first projection's operands land first
            for c0, c1 in ((0, 1024), (1024, 2048), (2048, 4096)):
                for t in ("k", "q"):
                    dma_w(t, c0, c1)
                    dma_a(t, 0, c0, c1)
            for c0, c1 in ((0, 2048), (2048, 4096)):
                dma_w("v", c0, c1)
                dma_a("v", 0, c0, c1)

        def ensure_loaded(tname, lb):
            dma_w(tname, 0, 2048)
            dma_w(tname, 2048, 4096)
            dma_a(tname, lb, 0, 2048)
            dma_a(tname, lb, 2048, 4096)

        def proj_half(tname, lb, i, half):
            """One half (4 of 8 d-steps) of a projection part.
            Only one accumulation chain may be open per PSUM bank, so before
            opening a new part close any half-open one (emit its 2nd half)."""
            if half == 0 and state.get("open_part") not in (None, (tname, lb, i)):
                op = state["open_part"]
                if (op, 1) in fillq:
                    fillq.remove((op, 1))
                    proj_half(*op, 1)
            state["open_part"] = (tname, lb, i) if half == 0 else None
            ensure_loaded(tname, lb)
            act = get_act_tile(tname, lb)
            wt = w_t[tname]
            pkey = ("pj", tname, lb, i)
            if half == 0:
                state[pkey] = ps.tile([128, 512], F32, tag="pj", bufs=2,
                                      name=f"pj{tname}{lb}{i}")
            p = state[pkey]
            for d in range(4 * half, 4 * half + 4):
                if tname != "v":
                    hp = i
                    nc.tensor.matmul(p[:],
                                     wt[:, d * 512 + hp * 128:d * 512 + (hp + 1) * 128],
                                     act[:, d * 512:(d + 1) * 512],
                                     start=(d == 0), stop=(d == 7),
                                     skip_group_check=True)
                else:
                    ls = i
                    nc.tensor.matmul(p[:],
                                     act[:, d * 512 + ls * 128:d * 512 + ls * 128 + 128],
                                     wt[:, d * 512:(d + 1) * 512],
                                     start=(d == 0), stop=(d == 7),
                                     skip_group_check=True)
            if half == 1:
                if tname != "v":
                    dst = qwT if tname == "q" else kwT
                    nc.vector.tensor_copy(dst[i][:, lb * 512:(lb + 1) * 512], p[:])
                else:
                    u = lb * 4 + i
                    v3d = vw_aug[u][:].rearrange("p (h c) -> p h c", h=NH)
                    nc.vector.tensor_copy(
                        v3d[:, :, 0:64], p[:].rearrange("p (h c) -> p h c", h=NH))
                    nc.vector.tensor_copy(
                        v3d[:, :, 64:65].squeeze(2),
                        vmask_sb[:, u:u + 1].broadcast_to([128, NH]))

        # ---- filler queue: (t, lb, i, half) in plan order ----
        fillq = []

        def chunk_needs(tau, h, ulo, uhi, part):
            # only S-operand (q/k) deps gate a chunk's emission; v parts are
            # drained lazily right before the PV closures that read them
            t, lb, i = part
            if t == "q":
                return lb == tau and i == h // 2
            if t == "k":
                return i == h // 2 and ulo // 4 <= lb <= uhi // 4
            return False

        def drain_deps(tau, h, ulo, uhi):
            rest = []
            for part, half in fillq:
                if chunk_needs(tau, h, ulo, uhi, part):
                    proj_half(*part, half)
                else:
                    rest.append((part, half))
            fillq[:] = rest

        def drain_v(lb_lo, lb_hi):
            rest = []
            for part, half in fillq:
                if part[0] == "v" and lb_lo <= part[1] <= lb_hi:
                    proj_half(*part, half)
                else:
                    rest.append((part, half))
            fillq[:] = rest

        def pop_filler(allow_v=True):
            # during the startup DMA crunch, v matmuls may stall on their
            # input DMA and would block later S matmuls in the PE queue
            for idx, (part, half) in enumerate(fillq):
                if allow_v or part[0] != "v":
                    fillq.pop(idx)
                    proj_half(*part, half)
                    return

        # pending PV / spill / finalize closures.  Dripped out a few per exp
        # batch so the PE stream always has S matmuls issued ahead of work
        # that blocks on an exp result (or on a pending input DMA).
        pending = []

        def flush_pending(n=None):
            cnt = len(pending) if n is None else n
            while pending and cnt > 0:
                pending.pop(0)()
                cnt -= 1

        def attn_chunk(tau, h, ulo, uhi):
            """Attention for q-tile tau, head h, k-subtiles ulo..uhi.
            Accumulates into a psum tile; spills to o_acc[tau] SBUF unless
            this is the only chunk for (tau, h)."""
            hp, half = h // 2, h % 2
            drain_deps(tau, h, ulo, uhi)
            whole = (ulo == 0 and uhi == 4 * tau + 3)
            # late (Act-heavy) waves get a slower filler drip so some proj
            # work is left to keep the PE warm at the end of the kernel
            late = ulo >= 8
            cell = {}

            def get_ov(cell=cell, tau=tau, h=h, ulo=ulo):
                if "ov" not in cell:
                    cell["ov"] = ps.tile([128, 512], F32, tag="ov", bufs=2,
                                         name=f"ov{tau}{h}{ulo}")
                return cell["ov"]
            for b in range((uhi - ulo + 1) // 2):
                u0 = ulo + 2 * b
                diag = u0 >= 4 * tau
                s = ps.tile([128, 1024], F32, tag="sps", bufs=2,
                            name=f"s{tau}{h}{u0}")
                for j in range(2):
                    u = u0 + j
                    col0 = 128 * max(0, u - 4 * tau)
                    nc.tensor.matmul(
                        s[:, j * 512 + col0:(j + 1) * 512],
                        kwT[hp][64 * half:64 * half + 64, u * 128:(u + 1) * 128],
                        qwT[hp][64 * half:64 * half + 64,
                                tau * 512 + col0:(tau + 1) * 512],
                        start=True, stop=True, skip_group_check=True,
                        tile_position=(64 * half, 0))
                pt = sb.tile([128, 1024], BF16, tag="pT", bufs=8,
                             name=f"pt{tau}{h}{u0}")
                if diag and u0 == 4 * tau:
                    # first diag batch: col0 = 0 / 128; one full-width exp is
                    # cheaper than two slices (cols [512,640) are garbage that
                    # the PV loop never reads)
                    nc.scalar.activation(pt[:], s[:],
                                         mybir.ActivationFunctionType.Exp,
                                         scale=0.125)
                    for j in range(2):
                        col0 = 128 * j
                        msl = slice(j * 512 + col0, j * 512 + col0 + 128)
                        nc.vector.tensor_mul(pt[:, msl], pt[:, msl], tri[:])
                elif diag:
                    for j in range(2):
                        u = u0 + j
                        col0 = 128 * (u - 4 * tau)
                        sl = slice(j * 512 + col0, (j + 1) * 512)
                        nc.scalar.activation(pt[:, sl], s[:, sl],
                                             mybir.ActivationFunctionType.Exp,
                                             scale=0.125)
                        msl = slice(j * 512 + col0, j * 512 + col0 + 128)
                        nc.vector.tensor_mul(pt[:, msl], pt[:, msl], tri[:])
                else:
                    nc.scalar.activation(pt[:], s[:],
                                         mybir.ActivationFunctionType.Exp,
                                         scale=0.125)
                if late:
                    flush_pending(1)
                elif tau > 0:
                    flush_pending(2)
                elif h >= 4:
                    flush_pending(1)
                if not late or b % 3 == 0:
                    pop_filler(allow_v=(tau > 0 or h >= 4))

                # PSUM allows only ONE open accumulation chain per bank:
                # the 4 qb-chains into ov must be emitted back to back, so
                # PV for the whole chunk goes out as one closure once the
                # last batch's exp has been issued.
                cell.setdefault("pts", []).append((pt, u0))
                if u0 + 1 == uhi:
                    def pv(pts=cell["pts"], tau=tau, h=h, ulo=ulo, uhi=uhi,
                           get_ov=get_ov):
                        drain_v(ulo // 4, uhi // 4)
                        ov = get_ov()
                        for qb in range(4):
                            for pt, u0 in pts:
                                for j in range(2):
                                    u = u0 + j
                                    if qb < u - 4 * tau or u > 4 * tau + qb:
                                        continue
                                    nc.tensor.matmul(
                                        ov[:, qb * 65:(qb + 1) * 65],
                                        pt[:, j * 512 + qb * 128:j * 512 + (qb + 1) * 128],
                                        vw_aug[u][:, h * 65:(h + 1) * 65],
                                        start=(u == ulo),
                                        stop=(u == min(uhi, 4 * tau + qb)),
                                        skip_group_check=True)
                    pending.append(pv)

            if not whole:
                key = ("oacc", tau)
                if key not in state:
                    state[key] = sb.tile([128, NH * 260], F32, tag="oacc",
                                         bufs=3, name=f"oacc{tau}")
                o_acc = state[key]
                dst = o_acc[:, h * 260:(h + 1) * 260]

                def spill(dst=dst, get_ov=get_ov, first=(ulo == 0)):
                    ov = get_ov()
                    if first:
                        nc.vector.tensor_copy(dst, ov[:, 0:260])
                    else:
                        nc.vector.tensor_add(dst, dst, ov[:, 0:260])
                pending.append(spill)

            if uhi == 4 * tau + 3:
                from_acc = not whole

                def finalize(from_acc=from_acc, get_ov=get_ov, tau=tau, h=h):
                    if from_acc:
                        src, base = state[("oacc", tau)], h * 260
                    else:
                        src, base = get_ov(), 0
                    oo = get_oo(tau)
                    rc = sb.tile([128, 4], F32, tag="rc", bufs=4,
                                 name=f"rc{tau}{h}")
                    nc.vector.reciprocal(rc[:], src[:, base + 64:base + 260:65])
                    for qb in range(4):
                        nc.vector.tensor_scalar_mul(
                            oo[:, qb * COLS + h * 64:qb * COLS + (h + 1) * 64],
                            src[:, base + qb * 65:base + qb * 65 + 64],
                            rc[:, qb:qb + 1])
                    # per-head store: one 3-d DMA, [512 rows x 64 cols] of out
                    src_ap = oo[:].rearrange("p (qs c) -> p qs c", qs=4)
                    nc.sync.dma_start(
                        out[tau * 512:(tau + 1) * 512, h * 64:(h + 1) * 64]
                        .rearrange("(qs p) c -> p qs c", qs=4),
                        src_ap[:, :, h * 64:(h + 1) * 64])
                pending.append(finalize)

        for _rep in range(reps):
            state.clear()
            for op in plan:
                if op[0] == "pre":
                    prologue_dma()
                elif op[0] == "load":
                    ensure_loaded(op[1], op[2])
                elif op[0] == "proj":
                    fillq.append(((op[1], op[2], op[3]), 0))
                    fillq.append(((op[1], op[2], op[3]), 1))
                elif op[0] == "attn":
                    attn_chunk(op[1], op[2], op[3], op[4])
            flush_pending()
            while fillq:
                pop_filler()

    nc.compile()
    return nc


_NC_CACHE = None


def _get_nc():
    global _NC_CACHE
    if _NC_CACHE is None:
        _NC_CACHE = _build_kernel()
    return _NC_CACHE


def _tile_act(x):
    """[2048, 1024] fp32 -> [512, 4096] bf16 with [lb*128+p, d*512+c] layout."""
    t = x.reshape(4, 512, 8, 128).transpose(0, 3, 2, 1)  # [lb, p, d, c]
    return np.ascontiguousarray(t.reshape(512, 4096).astype(BF16_NP))


def _tile_w(w):
    """[1024, 512] fp32 -> [128, 4096] bf16 with [p, d*512+c] layout."""
    t = w.reshape(8, 128, 512).transpose(1, 0, 2)  # [p, d, c]
    return np.ascontiguousarray(t.reshape(128, 4096).astype(BF16_NP))


def make_in_maps(q, k, v, v_mask, q_mask, Wq, Wk, Wv):
    q = np.asarray(q, np.float32)
    k = np.asarray(k, np.float32)
    v = np.asarray(v, np.float32)
    v_mask = np.asarray(v_mask, np.float32)
    Wq = np.asarray(Wq, np.float32)
    Wk = np.asarray(Wk, np.float32)
    Wv = np.asarray(Wv, np.float32)
    in_maps = []
    for core in range(8):
        b, g = core // 2, core % 2
        cs = slice(g * COLS, (g + 1) * COLS)
        vp = v[b] * v_mask[b][:, None]
        in_maps.append({
            "qTt": _tile_act(q[b]),
            "kTt": _tile_act(k[b]),
            "vTt": _tile_act(vp),
            "wq": _tile_w(Wq[:, cs]),
            "wk": _tile_w(Wk[:, cs]),
            "wv": _tile_w(Wv[:, cs]),
            "vmask_t": np.ascontiguousarray(v_mask[b].reshape(NKSUB, 128).T),
        })
    return in_maps


def kernel(q, k, v, v_mask, q_mask, Wq, Wk, Wv):
    nc = _get_nc()
    in_maps = make_in_maps(q, k, v, v_mask, q_mask, Wq, Wk, Wv)
    res = run_bass_kernel_spmd(nc, in_maps, core_ids=list(range(8)))
    q_mask = np.asarray(q_mask, np.float32)
    out = np.empty((4, L, 2 * COLS), np.float32)
    for core in range(8):
        b, g = core // 2, core % 2
        out[b, :, g * COLS:(g + 1) * COLS] = res.results[core]["out"]
    out *= q_mask[:, :, None]
    return out


# revision 26
# speedup vs baseline: 1.1513x; 1.1513x over previous
"""Multi-head causal attention (B=4, L=2048, D=1024, H=16, dh=64) on 8 TRN2 NeuronCores.

Sharding: core i handles batch b = i//2 and head-group g = i%2 (8 heads each).
No cross-core communication needed: each core computes o[b, :, g*512:(g+1)*512].

Per-core dataflow (all layouts chosen so matmul contraction is on partitions):
  inputs (host-prepared, bf16, tiled so every DMA is a single contiguous ~1MB read):
    qTt/kTt/vTt [512, 4096]: row lb*128+p, col d*512+c  holds  x[b][lb*512+c, d*128+p]
    wq/wk/wv    [128, 4096]: row p,       col d*512+c  holds  W[d*128+p, c]
  projections (bf16 matmuls, fp32 psum):
    qwT/kwT [128(2 heads x 64dh), L] bf16;  vw_aug [128(Lk sub), 8*65] bf16 with a
    v_mask column appended per head (gives sum-of-exp for free in the PV matmul).
  attention, per (q-tile tau of 512, head h, k-chunk of 4 subtiles):
    S^T[k,q] = kwT.T @ qwT per 128-k subtile (K=64 on the PE),
    P^T = exp(S^T/8) via ScalarE (psum->sbuf, bf16), causal zeroing of the
    partial 128x128 diagonal triangle via a DVE multiply with one shared
    triangular mask, then O[q, 65] += P^T-block.T @ vw_aug accumulated over
    k-subtiles directly in [q-partition, head-dim] orientation (col 64 = sum
    of exp).  No PE transposes needed; rows are scaled by 1/sumexp on DVE.
    k-chunks of a q-tile spill partial sums to SBUF (flash-attention style)
    so chunks can be scheduled early, keeping the exp load on ScalarE flat
    across the whole kernel while projections fill the PE slack.
v_mask is pre-applied to v on host (and to the ones column via vmask_t on device);
q_mask is applied to the returned output on host.  Masks are {0,1} so this is exact.
"""
import numpy as np
import ml_dtypes
from contextlib import ExitStack

import concourse.bass as bass
import concourse.tile as tile
from concourse import bacc, mybir
from concourse.bass_utils import run_bass_kernel_spmd

F32 = mybir.dt.float32
BF16 = mybir.dt.bfloat16
BF16_NP = ml_dtypes.bfloat16

L = 2048          # sequence length
D = 1024          # d_model
COLS = 512        # projection columns per core (8 heads x 64)
NKSUB = L // 128  # 16 k-subtiles
NTAU = L // 512   # 4 q-tiles
NH = 8            # heads per core


def _default_plan():
    """Emission plan: list of ops.
    ('pre',)                    startup DMA sequence (interleaved halves)
    ('proj', t, lb, i)          queue one projection part (2x4 matmuls)
    ('attn', tau, h, ulo, uhi)  one attention chunk (k-subtiles ulo..uhi)
    ('store', tau)              DMA the finished q-tile out

    Proj parts are queued; each attention chunk first emits any queued parts
    it depends on, then drips the rest into its batch slots (half a part per
    exp batch) so the PE stream never blocks the S matmuls that feed ScalarE.
    """
    def P(t, lb, i):
        return ("proj", t, lb, i)

    plan = [("pre",), P("k", 0, 0), P("q", 0, 0),
            P("v", 0, 0), P("v", 0, 1), P("v", 0, 2), P("v", 0, 3),
            ("load", "q", 1)]

    def wave(tau, ulo, uhi, pre=None):
        for h in range(NH):
            if pre and h in pre:
                plan.extend(pre[h])
            plan.append(("attn", tau, h, ulo, uhi))

    wave(0, 0, 3, pre={2: [P("k", 0, 1), P("q", 0, 1)],
                       3: [("load", "q", 2)],
                       4: [P("k", 0, 2), P("q", 0, 2)],
                       6: [P("k", 0, 3), P("q", 0, 3)],
                       7: [("load", "k", 1), ("load", "v", 1)]})
    wave(1, 0, 3, pre={h: [P("q", 1, h // 2)] for h in (0, 2, 4, 6)})
    plan.append(("load", "q", 3))
    wave(2, 0, 3, pre={h: [P("q", 2, h // 2)] for h in (0, 2, 4, 6)})
    plan.append(("load", "k", 2))
    wave(3, 0, 3, pre={0: [P("q", 3, 0), P("v", 1, 0)],
                       2: [P("q", 3, 1), P("v", 1, 1)],
                       4: [P("q", 3, 2), P("v", 1, 2)],
                       6: [P("q", 3, 3), P("v", 1, 3)]})
    wave(1, 4, 7, pre={0: [P("k", 1, 0)],
                       2: [P("k", 1, 1), ("load", "v", 2)],
                       4: [P("k", 1, 2)],
                       6: [P("k", 1, 3)]})
    wave(2, 4, 7, pre={0: [P("k", 2, 0)],
                       2: [P("k", 2, 1), ("load", "k", 3)],
                       4: [P("k", 2, 2)],
                       6: [P("k", 2, 3)]})
    wave(3, 4, 7, pre={h: [P("v", 2, h // 2)] for h in (0, 2, 4, 6)})
    wave(2, 8, 11, pre={0: [P("k", 3, 0)],
                        2: [P("k", 3, 1), ("load", "v", 3)],
                        4: [P("k", 3, 2)],
                        6: [P("k", 3, 3)]})
    wave(3, 8, 11, pre={0: [P("v", 3, 0), P("v", 3, 1)],
                        4: [P("v", 3, 2), P("v", 3, 3)]})
    wave(3, 12, 15)
    return plan


def _build_kernel(reps=1, plan=None):
    if plan is None:
        plan = _default_plan()
    nc = bacc.Bacc("TRN2", target_bir_lowering=False, debug=False, num_devices=8)

    qTt = nc.dram_tensor("qTt", [512, 4096], BF16, kind="ExternalInput").ap()
    kTt = nc.dram_tensor("kTt", [512, 4096], BF16, kind="ExternalInput").ap()
    vTt = nc.dram_tensor("vTt", [512, 4096], BF16, kind="ExternalInput").ap()
    wq = nc.dram_tensor("wq", [128, 4096], BF16, kind="ExternalInput").ap()
    wk = nc.dram_tensor("wk", [128, 4096], BF16, kind="ExternalInput").ap()
    wv = nc.dram_tensor("wv", [128, 4096], BF16, kind="ExternalInput").ap()
    vmask_t = nc.dram_tensor("vmask_t", [128, NKSUB], F32, kind="ExternalInput").ap()
    out = nc.dram_tensor("out", [L, COLS], F32, kind="ExternalOutput").ap()

    w_src = {"q": wq, "k": wk, "v": wv}
    a_src = {"q": qTt, "k": kTt, "v": vTt}

    with tile.TileContext(nc) as tc, ExitStack() as ctx:
        sb = ctx.enter_context(tc.tile_pool(name="sb", bufs=1))
        ps = ctx.enter_context(tc.tile_pool(name="ps", bufs=1, space="PSUM"))

        # ---- persistent SBUF tensors ----
        w_t = {t: sb.tile([128, 4096], BF16, tag="w", bufs=3, name=f"w{t}")
               for t in ("q", "k", "v")}

        vmask_sb = sb.tile([128, NKSUB], F32, tag="vm")
        nc.sync.dma_start(vmask_sb[:], vmask_t[:])

        # shared causal triangle for the partial diagonal blocks:
        # tri[p, c] = 1 if c >= p else 0   (query col >= key partition)
        tri = sb.tile([128, 128], BF16, tag="tri")
        nc.gpsimd.memset(tri[:], 1.0)
        nc.gpsimd.affine_select(
            out=tri[:], in_=tri[:], compare_op=mybir.AluOpType.is_ge, fill=0.0,
            base=0, channel_multiplier=-1, pattern=[[1, 128]])

        qwT = [sb.tile([128, L], BF16, tag="qwT", bufs=4, name=f"qwT{hp}")
               for hp in range(4)]
        kwT = [sb.tile([128, L], BF16, tag="kwT", bufs=4, name=f"kwT{hp}")
               for hp in range(4)]
        vw_aug = [sb.tile([128, NH * 65], BF16, tag="vwa", bufs=NKSUB,
                          name=f"vwa{u}") for u in range(NKSUB)]

        state = {}

        def get_oo(tau):
            key = ("oo", tau)
            if key not in state:
                state[key] = sb.tile([128, 4 * COLS], F32, tag="oo", bufs=2,
                                     name=f"oo{tau}")
            return state[key]

        def dma_w(tname, c0, c1):
            key = ("w", tname, c0)
            if key not in state:
                state[key] = True
                nc.sync.dma_start(w_t[tname][:, c0:c1], w_src[tname][:, c0:c1])

        def dma_a(tname, lb, c0, c1):
            key = ("adma", tname, lb, c0)
            if key in state:
                return
            state[key] = True
            a = get_act_tile(tname, lb)
            nc.sync.dma_start(a[:, c0:c1],
                              a_src[tname][lb * 128:(lb + 1) * 128, c0:c1])

        def get_act_tile(tname, lb):
            key = ("act", tname, lb)
            if key not in state:
                state[key] = sb.tile([128, 4096], BF16, tag="act", bufs=6,
                                     name=f"a{tname}{lb}")
            return state[key]

        def prologue_dma():
            # interleave so the first projection's operands land first
            for c0, c1 in ((0, 1024), (1024, 2048), (2048, 4096)):
                for t in ("k", "q"):
                    dma_w(t, c0, c1)
                    dma_a(t, 0, c0, c1)
            for c0, c1 in ((0, 2048), (2048, 4096)):
                dma_w("v", c0, c1)
                dma_a("v", 0, c0, c1)

        def ensure_loaded(tname, lb):
            dma_w(tname, 0, 2048)
            dma_w(tname, 2048, 4096)
            dma_a(tname, lb, 0, 2048)
            dma_a(tname, lb, 2048, 4096)

        def proj_half(tname, lb, i, half):
            """One half (4 of 8 d-steps) of a projection part.
            Only one accumulation chain may be open per PSUM bank, so before
            opening a new part close any half-open one (emit its 2nd half)."""
            if half == 0 and state.get("open_part") not in (None, (tname, lb, i)):
                op = state["open_part"]
                if (op, 1) in fillq:
                    fillq.remove((op, 1))
                    proj_half(*op, 1)
            state["open_part"] = (tname, lb, i) if half == 0 else None
            ensure_loaded(tname, lb)
            act = get_act_tile(tname, lb)
            wt = w_t[tname]
            pkey = ("pj", tname, lb, i)
            if half == 0:
                state[pkey] = ps.tile([128, 512], F32, tag="pj", bufs=2,
                                      name=f"pj{tname}{lb}{i}")
            p = state[pkey]
            for d in range(4 * half, 4 * half + 4):
                if tname != "v":
                    hp = i
                    nc.tensor.matmul(p[:],
                                     wt[:, d * 512 + hp * 128:d * 512 + (hp + 1) * 128],
                                     act[:, d * 512:(d + 1) * 512],
                                     start=(d == 0), stop=(d == 7),
                                     skip_group_check=True)
                else:
                    ls = i
                    nc.tensor.matmul(p[:],
                                     act[:, d * 512 + ls * 128:d * 512 + ls * 128 + 128],
                                     wt[:, d * 512:(d + 1) * 512],
                                     start=(d == 0), stop=(d == 7),
                                     skip_group_check=True)
            if half == 1:
                if tname != "v":
                    dst = qwT if tname == "q" else kwT
                    nc.vector.tensor_copy(dst[i][:, lb * 512:(lb + 1) * 512], p[:])
                else:
                    u = lb * 4 + i
                    v3d = vw_aug[u][:].rearrange("p (h c) -> p h c", h=NH)
                    nc.vector.tensor_copy(
                        v3d[:, :, 0:64], p[:].rearrange("p (h c) -> p h c", h=NH))
                    nc.vector.tensor_copy(
                        v3d[:, :, 64:65].squeeze(2),
                        vmask_sb[:, u:u + 1].broadcast_to([128, NH]))

        # ---- filler queue: (t, lb, i, half) in plan order ----
        fillq = []

        def chunk_needs(tau, h, ulo, uhi, part):
            # only S-operand (q/k) deps gate a chunk's emission; v parts are
            # drained lazily right before the PV closures that read them
            t, lb, i = part
            if t == "q":
                return lb == tau and i == h // 2
            if t == "k":
                return i == h // 2 and ulo // 4 <= lb <= uhi // 4
            return False

        def drain_deps(tau, h, ulo, uhi):
            rest = []
            for part, half in fillq:
                if chunk_needs(tau, h, ulo, uhi, part):
                    proj_half(*part, half)
                else:
                    rest.append((part, half))
            fillq[:] = rest

        def drain_v(lb_lo, lb_hi):
            rest = []
            for part, half in fillq:
                if part[0] == "v" and lb_lo <= part[1] <= lb_hi:
                    proj_half(*part, half)
                else:
                    rest.append((part, half))
            fillq[:] = rest

        def pop_filler(allow_v=True):
            # during the startup DMA crunch, v matmuls may stall on their
            # input DMA and would block later S matmuls in the PE queue
            for idx, (part, half) in enumerate(fillq):
                if allow_v or part[0] != "v":
                    fillq.pop(idx)
                    proj_half(*part, half)
                    return

        # pending PV / spill / finalize closures.  Dripped out a few per exp
        # batch so the PE stream always has S matmuls issued ahead of work
        # that blocks on an exp result (or on a pending input DMA).
        pending = []

        def flush_pending(n=None):
            cnt = len(pending) if n is None else n
            while pending and cnt > 0:
                pending.pop(0)()
                cnt -= 1

        def attn_chunk(tau, h, ulo, uhi):
            """Attention for q-tile tau, head h, k-subtiles ulo..uhi.
            Accumulates into a psum tile; spills to o_acc[tau] SBUF unless
            this is the only chunk for (tau, h)."""
            hp, half = h // 2, h % 2
            drain_deps(tau, h, ulo, uhi)
            whole = (ulo == 0 and uhi == 4 * tau + 3)
            # late (Act-heavy) waves get a slower filler drip so some proj
            # work is left to keep the PE warm at the end of the kernel
            late = ulo >= 8
            cell = {}

            def get_ov(cell=cell, tau=tau, h=h, ulo=ulo):
                if "ov" not in cell:
                    cell["ov"] = ps.tile([128, 512], F32, tag="ov", bufs=2,
                                         name=f"ov{tau}{h}{ulo}")
                return cell["ov"]
            for b in range((uhi - ulo + 1) // 2):
                u0 = ulo + 2 * b
                diag = u0 >= 4 * tau
                s = ps.tile([128, 1024], F32, tag="sps", bufs=2,
                            name=f"s{tau}{h}{u0}")
                for j in range(2):
                    u = u0 + j
                    col0 = 128 * max(0, u - 4 * tau)
                    nc.tensor.matmul(
                        s[:, j * 512 + col0:(j + 1) * 512],
                        kwT[hp][64 * half:64 * half + 64, u * 128:(u + 1) * 128],
                        qwT[hp][64 * half:64 * half + 64,
                                tau * 512 + col0:(tau + 1) * 512],
                        start=True, stop=True, skip_group_check=True,
                        tile_position=(64 * half, 0))
                pt = sb.tile([128, 1024], BF16, tag="pT", bufs=8,
                             name=f"pt{tau}{h}{u0}")
                if diag:
                    for j in range(2):
                        u = u0 + j
                        col0 = 128 * (u - 4 * tau)
                        sl = slice(j * 512 + col0, (j + 1) * 512)
                        nc.scalar.activation(pt[:, sl], s[:, sl],
                                             mybir.ActivationFunctionType.Exp,
                                             scale=0.125)
                        msl = slice(j * 512 + col0, j * 512 + col0 + 128)
                        nc.vector.tensor_mul(pt[:, msl], pt[:, msl], tri[:])
                else:
                    nc.scalar.activation(pt[:], s[:],
                                         mybir.ActivationFunctionType.Exp,
                                         scale=0.125)
                if late:
                    flush_pending(1)
                elif tau > 0:
                    flush_pending(2)
                elif h >= 4:
                    flush_pending(1)
                if not late or b % 3 == 0:
                    pop_filler(allow_v=(tau > 0 or h >= 4))

                # PSUM allows only ONE open accumulation chain per bank:
                # the 4 qb-chains into ov must be emitted back to back, so
                # PV for the whole chunk goes out as one closure once the
                # last batch's exp has been issued.
                cell.setdefault("pts", []).append((pt, u0))
                if u0 + 1 == uhi:
                    def pv(pts=cell["pts"], tau=tau, h=h, ulo=ulo, uhi=uhi,
                           get_ov=get_ov):
                        drain_v(ulo // 4, uhi // 4)
                        ov = get_ov()
                        for qb in range(4):
                            for pt, u0 in pts:
                                for j in range(2):
                                    u = u0 + j
                                    if qb < u - 4 * tau or u > 4 * tau + qb:
                                        continue
                                    nc.tensor.matmul(
                                        ov[:, qb * 65:(qb + 1) * 65],
                                        pt[:, j * 512 + qb * 128:j * 512 + (qb + 1) * 128],
                                        vw_aug[u][:, h * 65:(h + 1) * 65],
                                        start=(u == ulo),
                                        stop=(u == min(uhi, 4 * tau + qb)),
                                        skip_group_check=True)
                    pending.append(pv)

            if not whole:
                key = ("oacc", tau)
                if key not in state:
                    state[key] = sb.tile([128, NH * 260], F32, tag="oacc",
                                         bufs=3, name=f"oacc{tau}")
                o_acc = state[key]
                dst = o_acc[:, h * 260:(h + 1) * 260]

                def spill(dst=dst, get_ov=get_ov, first=(ulo == 0)):
                    ov = get_ov()
                    if first:
                        nc.vector.tensor_copy(dst, ov[:, 0:260])
                    else:
                        nc.vector.tensor_add(dst, dst, ov[:, 0:260])
                pending.append(spill)

            if uhi == 4 * tau + 3:
                from_acc = not whole

                def finalize(from_acc=from_acc, get_ov=get_ov, tau=tau, h=h):
                    if from_acc:
                        src, base = state[("oacc", tau)], h * 260
                    else:
                        src, base = get_ov(), 0
                    oo = get_oo(tau)
                    rc = sb.tile([128, 4], F32, tag="rc", bufs=4,
                                 name=f"rc{tau}{h}")
                    nc.vector.reciprocal(rc[:], src[:, base + 64:base + 260:65])
                    for qb in range(4):
                        nc.vector.tensor_scalar_mul(
                            oo[:, qb * COLS + h * 64:qb * COLS + (h + 1) * 64],
                            src[:, base + qb * 65:base + qb * 65 + 64],
                            rc[:, qb:qb + 1])
                    # per-head store: one 3-d DMA, [512 rows x 64 cols] of out
                    src_ap = oo[:].rearrange("p (qs c) -> p qs c", qs=4)
                    nc.sync.dma_start(
                        out[tau * 512:(tau + 1) * 512, h * 64:(h + 1) * 64]
                        .rearrange("(qs p) c -> p qs c", qs=4),
                        src_ap[:, :, h * 64:(h + 1) * 64])
                pending.append(finalize)

        for _rep in range(reps):
            state.clear()
            for op in plan:
                if op[0] == "pre":
                    prologue_dma()
                elif op[0] == "load":
                    ensure_loaded(op[1], op[2])
                elif op[0] == "proj":
                    fillq.append(((op[1], op[2], op[3]), 0))
                    fillq.append(((op[1], op[2], op[3]), 1))
                elif op[0] == "attn":
                    attn_chunk(op[1], op[2], op[3], op[4])
            flush_pending()
            while fillq:
                pop_filler()

    nc.compile()
    return nc


_NC_CACHE = None


def _get_nc():
    global _NC_CACHE
    if _NC_CACHE is None:
        _NC_CACHE = _build_kernel()
    return _NC_CACHE


def _tile_act(x):
    """[2048, 1024] fp32 -> [512, 4096] bf16 with [lb*128+p, d*512+c] layout."""
    t = x.reshape(4, 512, 8, 128).transpose(0, 3, 2, 1)  # [lb, p, d, c]
    return np.ascontiguousarray(t.reshape(512, 4096).astype(BF16_NP))


def _tile_w(w):
    """[1024, 512] fp32 -> [128, 4096] bf16 with [p, d*512+c] layout."""
    t = w.reshape(8, 128, 512).transpose(1, 0, 2)  # [p, d, c]
    return np.ascontiguousarray(t.reshape(128, 4096).astype(BF16_NP))


def make_in_maps(q, k, v, v_mask, q_mask, Wq, Wk, Wv):
    q = np.asarray(q, np.float32)
    k = np.asarray(k, np.float32)
    v = np.asarray(v, np.float32)
    v_mask = np.asarray(v_mask, np.float32)
    Wq = np.asarray(Wq, np.float32)
    Wk = np.asarray(Wk, np.float32)
    Wv = np.asarray(Wv, np.float32)
    in_maps = []
    for core in range(8):
        b, g = core // 2, core % 2
        cs = slice(g * COLS, (g + 1) * COLS)
        vp = v[b] * v_mask[b][:, None]
        in_maps.append({
            "qTt": _tile_act(q[b]),
            "kTt": _tile_act(k[b]),
            "vTt": _tile_act(vp),
            "wq": _tile_w(Wq[:, cs]),
            "wk": _tile_w(Wk[:, cs]),
            "wv": _tile_w(Wv[:, cs]),
            "vmask_t": np.ascontiguousarray(v_mask[b].reshape(NKSUB, 128).T),
        })
    return in_maps


def kernel(q, k, v, v_mask, q_mask, Wq, Wk, Wv):
    nc = _get_nc()
    in_maps = make_in_maps(q, k, v, v_mask, q_mask, Wq, Wk, Wv)
    res = run_bass_kernel_spmd(nc, in_maps, core_ids=list(range(8)))
    q_mask = np.asarray(q_mask, np.float32)
    out = np.empty((4, L, 2 * COLS), np.float32)
    for core in range(8):
        b, g = core // 2, core % 2
        out[b, :, g * COLS:(g + 1) * COLS] = res.results[core]["out"]
    out *= q_mask[:, :, None]
    return out
